# revision 13
# baseline (speedup 1.0000x reference)
"""GQA attention (RoPE + causal softmax + o_proj) on 8 Trainium2 NeuronCores.

Sharding: core = b*4 + g where b = batch (2), g = head-group (4).
Each core handles 8 query heads (global 8g..8g+7) and their 2 KV heads
(2g, 2g+1) for one batch element, producing a partial o_proj output
(contraction over its 512 of the 2048 hd dims). The host sums the 4
partials per batch element.

v3 schedule: projections for s-chunk 0 run dense at the head; the
projections for s-chunks 1-3 and the o_proj of chunk c-1 are fed into
chunk c's kb loop as PE filler quanta, so the PE (the overall pacer at
~226us of stream time) never waits on the ACT exp stream. PSUM is
partitioned into dedicated rings (scores 4 banks / AV 2 / filler 2) so
long-lived AV accumulators never block filler allocation. Softmax
normalization is emitted as quanta into the NEXT pg's kb loop, so the
AV-bank release (an SBUF staging copy) never stalls the PE.

Engine discipline learned from traces:
  - every dma_start costs ~600ns of serial Sync-engine issue time and
    the SP executes in emission order, so the bulk input load uses 9
    mega-DMAs with 3D access patterns (all 16 hid-chunks per
    instruction) instead of ~70 per-tile DMAs that head-block the
    rope-swap/normalize DMAs;
  - GPSIMD runs ONLY partition_broadcast: mixing other ops forces a
    MODIFY_POOL_CONFIG IRAM reload (~6us, invisible in profiles) per
    switch, which serialized every pg at ~12us in an earlier rev;
  - strided-partition DMA views silently corrupt (only one row per
    block lands) - the rope swap stays 4 plain 32-row DMAs;
  - a junk-matmul warmup burst at t=0 flips the PE HAM clock gate to
    2.4 GHz before the first real matmul group's DMA gate clears.
"""

import numpy as np
import ml_dtypes
from contextlib import ExitStack

import concourse.mybir as mybir
from concourse import bacc
from concourse.tile import TileContext
from concourse.bass_utils import run_bass_kernel_spmd

BF16 = mybir.dt.bfloat16
F32 = mybir.dt.float32
NP_BF16 = ml_dtypes.bfloat16

HID = 2048
D = 64
H = 32           # global query heads
KV = 8           # global kv heads
B = 2
P = 128
SC = 512         # q-chunk width (also matmul free dim / PSUM bank)

_CACHE = {}


def build_nc(S):
    assert S % SC == 0
    NHID = HID // P       # hid chunks (16)
    NSB = S // P          # 128-row s-blocks
    NSC = S // SC         # 512-col s-chunks
    EXP = mybir.ActivationFunctionType.Exp

    nc = bacc.Bacc("TRN2", target_bir_lowering=False, debug=False)
    xT = nc.dram_tensor("xT", [HID, S], BF16, kind="ExternalInput")
    # [q pairs (512) | k (128) | v (128)] merged per hid chunk
    wqkv = nc.dram_tensor("wqkv", [HID, 768], BF16, kind="ExternalInput")
    wo = nc.dram_tensor("wo", [512, HID], BF16, kind="ExternalInput")
    cosT = nc.dram_tensor("cosT", [128, S], BF16, kind="ExternalInput")
    sinT2 = nc.dram_tensor("sinT2", [128, S], BF16, kind="ExternalInput")
    trimask = nc.dram_tensor("trimask", [128, 128], BF16, kind="ExternalInput")
    o_part = nc.dram_tensor("o_part", [S, HID], BF16, kind="ExternalOutput")

    with TileContext(nc) as tc, ExitStack() as ctx:
        res = ctx.enter_context(tc.tile_pool(name="res", bufs=1))
        rope = ctx.enter_context(tc.tile_pool(name="rope", bufs=2))
        ptp = ctx.enter_context(tc.tile_pool(name="ptp", bufs=6))
        stg = ctx.enter_context(tc.tile_pool(name="stg", bufs=2))
        obp = ctx.enter_context(tc.tile_pool(name="obp", bufs=2))
        psum = ctx.enter_context(tc.tile_pool(name="psum", bufs=1, space="PSUM"))

        # ---- mega-tiles: all 16 hid-chunks side by side ----
        wqall = res.tile([P, NHID * 768], BF16, tag="wqall")
        xtall = res.tile([P, NHID * S], BF16, tag="xtall")

        def wsl(h, lo, hi):
            return wqall[:, h * 768 + lo:h * 768 + hi]

        def xsl(h, lo, hi):
            return xtall[:, h * S + lo:h * S + hi]

        wqv = wqall.rearrange("p (h c) -> p h c", c=768)
        wqs = wqkv.rearrange("(h p) c -> p h c", h=NHID)
        xtv = xtall.rearrange("p (h c) -> p h c", c=S)
        xts_ = xT.rearrange("(h p) c -> p h c", h=NHID)

        # ---- input staging: mega-DMAs ordered by first consumption;
        # k+v weight columns land first so the k-projection and v units
        # start as early as possible ----
        nc.sync.dma_start(out=wqv[:, :, 512:768], in_=wqs[:, :, 512:768])
        nc.sync.dma_start(out=xtv[:, :, 0:SC], in_=xts_[:, :, 0:SC])
        cos_sb = res.tile([P, S], BF16, tag="cos")
        nc.sync.dma_start(out=cos_sb, in_=cosT[:, :])
        sin2_sb = res.tile([P, S], BF16, tag="sin2")
        nc.sync.dma_start(out=sin2_sb, in_=sinT2[:, :])
        mask_sb = res.tile([P, P], BF16, tag="mask")
        nc.sync.dma_start(out=mask_sb, in_=trimask[:, :])
        nc.sync.dma_start(out=wqv[:, :, 0:512], in_=wqs[:, :, 0:512])
        nc.sync.dma_start(out=xtv[:, :, SC:2 * SC], in_=xts_[:, :, SC:2 * SC])
        wo_sb = []
        for i in range(4):
            t = res.tile([P, HID], BF16, tag=f"wo{i}", name=f"wo{i}")
            nc.sync.dma_start(out=t, in_=wo[i * P:(i + 1) * P, :])
            wo_sb.append(t)
        nc.sync.dma_start(out=xtv[:, :, 2 * SC:4 * SC], in_=xts_[:, :, 2 * SC:4 * SC])

        # ---- persistent compute tiles ----
        qkrot = [res.tile([P, S], BF16, tag=f"qkrot{m}", name=f"qkrot{m}")
                 for m in range(5)]
        # v tiles [128, 194]: [v0(0:64) | 1 | 0 | v1(66:130) | 1 | 0-pad] —
        # both AV stationaries are 128 columns (FWL) at 4B-aligned offsets;
        # the ones column puts the softmax denominator in PSUM row 64
        vnat = [res.tile([P, 194], BF16, tag=f"vnat{sb}", name=f"vnat{sb}")
                for sb in range(NSB)]
        attnT = [res.tile([P, S], BF16, tag=f"attnT{i}", name=f"attnT{i}")
                 for i in range(4)]

        # ---- PE warmup: flip the HAM clock gate to 8/8 while the input
        # DMAs stream; junk matmuls depend only on one vector memset ----
        junk = res.tile([P, 640], BF16, tag="junk")
        nc.vector.memset(junk, 0.01)
        with nc.named_scope("warmup"):
            for u in range(3):
                pj = psum.tile([P, SC], F32, tag="fill", bufs=2, name="pj")
                for r in range(16):
                    nc.tensor.matmul(
                        pj, lhsT=junk[:, 0:128], rhs=junk[:, 128:640],
                        start=(r == 0), stop=(r == 15),
                    )
                sink = rope.tile([1, SC], BF16, tag="sink", bufs=2, name="sink")
                nc.vector.tensor_copy(sink, pj[0:1, :])

        def gen_proj_schunk(s):
            """Emit s-chunk s projections + RoPE + v as units (yields).

            Unit order (k, q0, v x4, q1-q3) so an attention chunk's pg0
            dependencies (k, qkrot[0], vnat) complete first. Matmul
            groups stay consecutive and eviction chains (rope / v
            copies) run on DVE/DMA only, so cross-engine releases never
            head-of-line-block the PE stream for long."""

            def m_unit(m):
                ps = psum.tile([P, SC], F32, tag="fill", bufs=2, name="ps_proj")
                for h0 in (0, 8):
                    for h in range(h0, h0 + 8):
                        nc.tensor.matmul(
                            ps,
                            lhsT=wsl(h, m * P, (m + 1) * P),
                            rhs=xsl(h, s * SC, (s + 1) * SC),
                            start=(h == 0),
                            stop=(h == NHID - 1),
                        )
                    yield
                # RoPE: q*cos + swap(q*sin2) where sin2 pre-folds the
                # rotate-half signs; the +-32-partition swap must go
                # through DMA (engines are lane-locked)
                sl = slice(s * SC, (s + 1) * SC)
                t1 = rope.tile([P, SC], BF16, tag="t1", bufs=2, name="t1")
                nc.vector.tensor_mul(t1, ps, cos_sb[:, sl])
                tmp = rope.tile([P, SC], BF16, tag="tmp", bufs=2, name="tmp")
                nc.vector.tensor_mul(tmp, ps, sin2_sb[:, sl])
                tswp = rope.tile([P, SC], BF16, tag="tswp", bufs=2, name="tswp")
                for dst, src in ((0, 32), (32, 0), (64, 96), (96, 64)):
                    nc.sync.dma_start(
                        out=tswp[dst:dst + 32, :], in_=tmp[src:src + 32, :]
                    )
                nc.vector.tensor_add(qkrot[m][:, sl], t1, tswp)
                yield

            def v_unit(sb):
                t = vnat[sb]
                nc.vector.memset(t[:, 64:65], 1.0)
                nc.vector.memset(t[:, 65:66], 0.0)
                nc.vector.memset(t[:, 130:131], 1.0)
                nc.vector.memset(t[:, 131:194], 0.0)
                pv = psum.tile([P, 128], F32, tag="fill", bufs=2, name="ps_v")
                for h in range(NHID):
                    nc.tensor.matmul(
                        pv,
                        lhsT=xsl(h, sb * P, (sb + 1) * P),
                        rhs=wsl(h, 640, 768),
                        start=(h == 0),
                        stop=(h == NHID - 1),
                    )
                yield
                nc.vector.tensor_copy(t[:, 0:64], pv[:, 0:64])
                nc.vector.tensor_copy(t[:, 66:130], pv[:, 64:128])
                yield

            yield from m_unit(4)
            yield from m_unit(0)
            for sb in range(4 * s, 4 * s + 4):
                yield from v_unit(sb)
            for m in (1, 2, 3):
                yield from m_unit(m)

        odve = [True]

        def gen_o_chunk(c):
            for qb in range(4 * c, 4 * c + 4):
                ob = obp.tile([P, HID], BF16, tag="ob", name="ob")
                for n in range(HID // SC):
                    po = psum.tile([P, SC], F32, tag="fill", bufs=2, name="po")
                    for i in range(4):
                        nc.tensor.matmul(
                            po,
                            lhsT=attnT[i][:, qb * P:(qb + 1) * P],
                            rhs=wo_sb[i][:, n * SC:(n + 1) * SC],
                            start=(i == 0),
                            stop=(i == 3),
                        )
                    # alternate the PSUM->SBUF evict between DVE and ACT
                    # to balance the two engines' attention-phase load
                    if odve[0]:
                        nc.vector.tensor_copy(ob[:, n * SC:(n + 1) * SC], po)
                    else:
                        nc.scalar.copy(ob[:, n * SC:(n + 1) * SC], po)
                    odve[0] = not odve[0]
                    yield
                nc.sync.dma_start(out=o_part[qb * P:(qb + 1) * P, :], in_=ob)

        def gen_norm(av0, av1, pg, q0):
            """Normalize one pg: stage AV to SBUF (releases the av banks),
            reciprocal of the ones-column denominators (row 64), broadcast
            (the ONLY gpsimd op - keeps its IRAM kernel resident), scale on
            DVE. attnT rows 0-63 = head pg, 64-127 = head pg+4 (cross-
            partition move via DMA). Emitted as quanta inside the NEXT
            pg's kb loop so the av-bank release never stalls the PE."""
            st01 = stg.tile([65, 2 * SC], F32, tag="stage", name="st01")
            nc.vector.tensor_copy(st01[:, 0:SC], av0[0:65, :])
            yield
            nc.vector.tensor_copy(st01[:, SC:2 * SC], av1[0:65, :])
            yield
            den2 = stg.tile([1, 2 * SC], F32, tag="den", bufs=1, name="den2")
            nc.sync.dma_start(out=den2, in_=st01[64:65, :])
            rec = stg.tile([1, 2 * SC], F32, tag="rec", bufs=1, name="rec")
            nc.vector.reciprocal_approx_fast(rec, den2)
            yield
            rb = stg.tile([64, 2 * SC], F32, tag="rb", bufs=1, name="rb")
            nc.gpsimd.partition_broadcast(rb, rec)
            yield
            nc.vector.tensor_mul(
                attnT[pg][0:64, q0:q0 + SC], st01[0:64, 0:SC], rb[:, 0:SC]
            )
            yield
            hi = stg.tile([64, SC], BF16, tag="hi", name="hi")
            nc.vector.tensor_mul(hi, st01[0:64, SC:2 * SC], rb[:, SC:2 * SC])
            nc.sync.dma_start(out=attnT[pg][64:128, q0:q0 + SC], in_=hi)
            yield

        # ---- s-chunk 0 projections run dense (nothing to overlap yet) ----
        with nc.named_scope("projA0"):
            for _ in gen_proj_schunk(0):
                pass

        # ---- attention chunks; chunk c's kb loops consume proj-s(c+1)
        # and o_proj(c-1) units as PE filler so the PE never idles on the
        # ACT exp stream; proj units drain by chunk end (next chunk needs
        # their qkrot/vnat), o units may spill into the next chunk ----
        o_gen = None
        norm_q = []
        for c in range(NSC):
          with nc.named_scope(f"attn_c{c}"):
            q0 = c * SC
            nkb = 4 * c + 4
            p_gen = gen_proj_schunk(c + 1) if c + 1 < NSC else None
            # all pending normalizes belong to chunk c-1, whose attnT the
            # o_proj(c-1) units read: flush them before o units can be
            # emitted (they overlap the proj drain / chunk entry)
            for g in norm_q:
                for _ in g:
                    pass
            norm_q = []
            if c >= 1:
                o_prev = gen_o_chunk(c - 1)
                o_gen = o_prev if o_gen is None else _chain(o_gen, o_prev)
            take_p = [True]

            def take_filler(k=1):
                nonlocal p_gen, o_gen
                for _ in range(k):
                    if take_p[0] and p_gen is not None:
                        try:
                            next(p_gen)
                        except StopIteration:
                            p_gen = None
                    elif o_gen is not None:
                        try:
                            next(o_gen)
                        except StopIteration:
                            o_gen = None
                    elif p_gen is not None:
                        try:
                            next(p_gen)
                        except StopIteration:
                            p_gen = None
                    else:
                        return
                    take_p[0] = not take_p[0]

            for pg in (0, 1, 2, 3):
                av0 = psum.tile([P, SC], F32, tag="av", bufs=2, name="av0")
                av1 = psum.tile([P, SC], F32, tag="av", bufs=2, name="av1")

                def emit_av(kb, pt, vs):
                    nc.tensor.matmul(
                        av0[:, vs:SC],
                        lhsT=vnat[kb][:, 0:128],
                        rhs=pt[:, vs:SC],
                        start=(kb == 0), stop=(kb == nkb - 1),
                    )
                    nc.tensor.matmul(
                        av1[:, vs:SC],
                        lhsT=vnat[kb][:, 66:194],
                        rhs=pt[:, SC:2 * SC - vs],
                        start=(kb == 0), stop=(kb == nkb - 1),
                    )

                # software pipeline: AV(kb-3) is emitted after scores(kb),
                # giving each exp three iterations of cover
                pending = []
                for kb in range(nkb):
                    # spill the previous pg's normalize ops in here
                    while norm_q and norm_q[0] is not None:
                        try:
                            next(norm_q[0])
                            break
                        except StopIteration:
                            norm_q.pop(0)
                    vs = max(0, (kb - 4 * c) * P)  # first valid col in chunk
                    st = psum.tile([P, 2 * SC], F32, tag="st", bufs=2, name="st")
                    nc.tensor.matmul(
                        st[:, vs:SC],
                        lhsT=qkrot[4][0:64, kb * P:(kb + 1) * P],
                        rhs=qkrot[pg][0:64, q0 + vs:q0 + SC],
                        start=True, stop=True,
                    )
                    # head hp+4 written left-shifted by vs so the exp span
                    # [vs : 2SC-vs] is contiguous with no dead columns
                    nc.tensor.matmul(
                        st[:, SC:2 * SC - vs],
                        lhsT=qkrot[4][64:128, kb * P:(kb + 1) * P],
                        rhs=qkrot[pg][64:128, q0 + vs:q0 + SC],
                        start=True, stop=True,
                    )
                    if len(pending) >= 3:
                        emit_av(*pending.pop(0))
                    pt = ptp.tile([P, 2 * SC], BF16, tag="pt", name="pt")
                    nc.scalar.activation(
                        pt[:, vs:2 * SC - vs], st[:, vs:2 * SC - vs], EXP,
                        scale=0.125,
                    )
                    if kb - 4 * c >= 0:  # diagonal block: mask triangle
                        nc.vector.tensor_mul(
                            pt[:, vs:vs + P], pt[:, vs:vs + P], mask_sb
                        )
                        nc.vector.tensor_mul(
                            pt[:, SC:SC + P], pt[:, SC:SC + P], mask_sb
                        )
                    pending.append((kb, pt, vs))
                    take_filler(2)
                for pp in pending:
                    emit_av(*pp)
                    take_filler(1)
                norm_q.append(gen_norm(av0, av1, pg, q0))

            # drain remaining proj units + spilled normalizes
            while p_gen is not None or norm_q:
                if norm_q:
                    try:
                        next(norm_q[0])
                    except StopIteration:
                        norm_q.pop(0)
                if p_gen is not None:
                    try:
                        next(p_gen)
                    except StopIteration:
                        p_gen = None
                if c == NSC - 1 and p_gen is None:
                    # final chunk: flush all normalizes before the o tail
                    for g in norm_q:
                        for _ in g:
                            pass
                    norm_q = []
        # last chunk's o_proj tail (+ any o units that spilled over)
        if o_gen is not None:
            for _ in o_gen:
                pass
        for _ in gen_o_chunk(NSC - 1):
            pass

    nc.finalize()
    return nc


def _chain(g1, g2):
    yield from g1
    yield from g2


def prep_core_inputs(x, cos, sin, wq, wk, wv, wo, core, _shared={}):
    """Build the per-core input map (all host-side numpy)."""
    b, g = core // 4, core % 4
    S = x.shape[1]

    key = ("xT", b, id(x))
    if key not in _shared:
        _shared.clear() if len(_shared) > 8 else None
        _shared[key] = np.ascontiguousarray(x[b].T).astype(NP_BF16)
    xT = _shared[key]

    qcols = []
    for i in range(4):
        h0, h1 = 8 * g + i, 8 * g + i + 4
        qcols.append(wq[:, h0 * D:(h0 + 1) * D])
        qcols.append(wq[:, h1 * D:(h1 + 1) * D])
    kcols = wk[:, 2 * g * D:(2 * g + 2) * D]
    vcols = wv[:, 2 * g * D:(2 * g + 2) * D]
    wqkv_c = np.concatenate(qcols + [kcols, vcols], axis=1).astype(NP_BF16)
    worows = []
    for i in range(4):
        h0, h1 = 8 * g + i, 8 * g + i + 4
        worows.append(wo[h0 * D:(h0 + 1) * D, :])
        worows.append(wo[h1 * D:(h1 + 1) * D, :])
    wo_c = np.concatenate(worows, axis=0).astype(NP_BF16)

    cosT = np.tile(cos[:S].T, (2, 1)).astype(NP_BF16)
    # sin pre-arranged for multiply-then-swap rope:
    # tmp[j] = q[j]*sin2[j]; result[i] = tmp[swap(i)] needs
    # sin2 = [sin[D/2:], -sin[:D/2]] per 64-row head block
    sinT = sin[:S].T
    sinT2_h = np.concatenate([sinT[D // 2:], -sinT[:D // 2]], axis=0)
    sinT2 = np.tile(sinT2_h, (2, 1)).astype(NP_BF16)
    trimask = np.triu(np.ones((P, P), dtype=NP_BF16))

    return {
        "xT": xT, "wqkv": wqkv_c, "wo": wo_c,
        "cosT": cosT, "sinT2": sinT2, "trimask": trimask,
    }


def kernel(x, cos, sin, wq, wk, wv, wo):
    x = np.asarray(x)
    S = x.shape[1]
    assert x.shape == (B, S, HID)
    if S not in _CACHE:
        _CACHE[S] = build_nc(S)
    nc = _CACHE[S]
    in_maps = [
        prep_core_inputs(x, np.asarray(cos), np.asarray(sin), np.asarray(wq),
                         np.asarray(wk), np.asarray(wv), np.asarray(wo), core)
        for core in range(8)
    ]
    res = run_bass_kernel_spmd(nc, in_maps, core_ids=list(range(8)))
    out = np.zeros((B, S, HID), np.float32)
    for core in range(8):
        out[core // 4] += res.results[core]["o_part"].astype(np.float32)
    return out


# revision 19
# speedup vs baseline: 1.0032x; 1.0032x over previous
"""GQA attention (RoPE + causal softmax + o_proj) on 8 Trainium2 NeuronCores.

Sharding: core = b*4 + g where b = batch (2), g = head-group (4).
Each core handles 8 query heads (global 8g..8g+7) and their 2 KV heads
(2g, 2g+1) for one batch element, producing a partial o_proj output
(contraction over its 512 of the 2048 hd dims). The host sums the 4
partials per batch element.

v3 schedule: projections for s-chunk 0 run dense at the head; the
projections for s-chunks 1-3 and the o_proj of chunk c-1 are fed into
chunk c's kb loop as PE filler quanta, so the PE (the overall pacer at
~226us of stream time) never waits on the ACT exp stream. PSUM is
partitioned into dedicated rings (scores 4 banks / AV 2 / filler 2) so
long-lived AV accumulators never block filler allocation. Softmax
normalization is emitted as quanta into the NEXT pg's kb loop, so the
AV-bank release (an SBUF staging copy) never stalls the PE.

Engine discipline learned from traces:
  - every dma_start costs ~600ns of serial Sync-engine issue time and
    the SP executes in emission order, so the bulk input load uses 9
    mega-DMAs with 3D access patterns (all 16 hid-chunks per
    instruction) instead of ~70 per-tile DMAs that head-block the
    rope-swap/normalize DMAs;
  - GPSIMD runs ONLY partition_broadcast: mixing other ops forces a
    MODIFY_POOL_CONFIG IRAM reload (~6us, invisible in profiles) per
    switch, which serialized every pg at ~12us in an earlier rev;
  - strided-partition DMA views silently corrupt (only one row per
    block lands) - the rope swap stays 4 plain 32-row DMAs;
  - a junk-matmul warmup burst at t=0 flips the PE HAM clock gate to
    2.4 GHz before the first real matmul group's DMA gate clears.
"""

import numpy as np
import ml_dtypes
from contextlib import ExitStack

import concourse.mybir as mybir
from concourse import bacc
from concourse.tile import TileContext
from concourse.bass_utils import run_bass_kernel_spmd

BF16 = mybir.dt.bfloat16
F32 = mybir.dt.float32
NP_BF16 = ml_dtypes.bfloat16

HID = 2048
D = 64
H = 32           # global query heads
KV = 8           # global kv heads
B = 2
P = 128
SC = 512         # q-chunk width (also matmul free dim / PSUM bank)

_CACHE = {}


def build_nc(S):
    assert S % SC == 0
    NHID = HID // P       # hid chunks (16)
    NSB = S // P          # 128-row s-blocks
    NSC = S // SC         # 512-col s-chunks
    EXP = mybir.ActivationFunctionType.Exp

    nc = bacc.Bacc("TRN2", target_bir_lowering=False, debug=False)
    xT = nc.dram_tensor("xT", [HID, S], BF16, kind="ExternalInput")
    # [q pairs (512) | k (128) | v (128)] merged per hid chunk
    wqkv = nc.dram_tensor("wqkv", [HID, 768], BF16, kind="ExternalInput")
    wo = nc.dram_tensor("wo", [512, HID], BF16, kind="ExternalInput")
    cosT = nc.dram_tensor("cosT", [128, S], BF16, kind="ExternalInput")
    sinT2 = nc.dram_tensor("sinT2", [128, S], BF16, kind="ExternalInput")
    trimask = nc.dram_tensor("trimask", [128, 128], BF16, kind="ExternalInput")
    o_part = nc.dram_tensor("o_part", [S, HID], BF16, kind="ExternalOutput")

    with TileContext(nc) as tc, ExitStack() as ctx:
        res = ctx.enter_context(tc.tile_pool(name="res", bufs=1))
        rope = ctx.enter_context(tc.tile_pool(name="rope", bufs=2))
        ptp = ctx.enter_context(tc.tile_pool(name="ptp", bufs=6))
        stg = ctx.enter_context(tc.tile_pool(name="stg", bufs=2))
        obp = ctx.enter_context(tc.tile_pool(name="obp", bufs=2))
        psum = ctx.enter_context(tc.tile_pool(name="psum", bufs=1, space="PSUM"))

        # ---- mega-tiles: all 16 hid-chunks side by side ----
        wqall = res.tile([P, NHID * 768], BF16, tag="wqall")
        xtall = res.tile([P, NHID * S], BF16, tag="xtall")

        def wsl(h, lo, hi):
            return wqall[:, h * 768 + lo:h * 768 + hi]

        def xsl(h, lo, hi):
            return xtall[:, h * S + lo:h * S + hi]

        wqv = wqall.rearrange("p (h c) -> p h c", c=768)
        wqs = wqkv.rearrange("(h p) c -> p h c", h=NHID)
        xtv = xtall.rearrange("p (h c) -> p h c", c=S)
        xts_ = xT.rearrange("(h p) c -> p h c", h=NHID)

        # ---- input staging: mega-DMAs ordered by first consumption.
        # Each issuing ENGINE owns a hardware DGE queue (issue and
        # transfer serialize per queue), so the loads are split between
        # the sync and scalar queues for parallel transfer streams, and
        # runtime DMAs never queue behind bulk loads. ----
        nc.sync.dma_start(out=xtv[:, :, 0:SC], in_=xts_[:, :, 0:SC])
        nc.sync.dma_start(out=wqv[:, :, 0:128], in_=wqs[:, :, 0:128])
        nc.scalar.dma_start(out=wqv[:, :, 512:768], in_=wqs[:, :, 512:768])
        cos_sb = res.tile([P, S], BF16, tag="cos")
        nc.scalar.dma_start(out=cos_sb, in_=cosT[:, :])
        sin2_sb = res.tile([P, S], BF16, tag="sin2")
        nc.scalar.dma_start(out=sin2_sb, in_=sinT2[:, :])
        mask_sb = res.tile([P, P], BF16, tag="mask")
        nc.scalar.dma_start(out=mask_sb, in_=trimask[:, :])
        nc.scalar.dma_start(out=wqv[:, :, 128:512], in_=wqs[:, :, 128:512])
        nc.scalar.dma_start(out=xtv[:, :, SC:2 * SC], in_=xts_[:, :, SC:2 * SC])
        wo_sb = []
        for i in range(4):
            t = res.tile([P, HID], BF16, tag=f"wo{i}", name=f"wo{i}")
            nc.scalar.dma_start(out=t, in_=wo[i * P:(i + 1) * P, :])
            wo_sb.append(t)
        nc.scalar.dma_start(out=xtv[:, :, 2 * SC:4 * SC], in_=xts_[:, :, 2 * SC:4 * SC])

        # ---- persistent compute tiles ----
        qkrot = [res.tile([P, S], BF16, tag=f"qkrot{m}", name=f"qkrot{m}")
                 for m in range(5)]
        # v tiles [128, 194]: [v0(0:64) | 1 | 0 | v1(66:130) | 1 | 0-pad] —
        # both AV stationaries are 128 columns (FWL) at 4B-aligned offsets;
        # the ones column puts the softmax denominator in PSUM row 64
        vnat = [res.tile([P, 194], BF16, tag=f"vnat{sb}", name=f"vnat{sb}")
                for sb in range(NSB)]
        attnT = [res.tile([P, S], BF16, tag=f"attnT{i}", name=f"attnT{i}")
                 for i in range(4)]

        # ---- PE warmup: flip the HAM clock gate to 8/8 while the input
        # DMAs stream; junk matmuls depend only on one vector memset ----
        junk = res.tile([P, 640], BF16, tag="junk")
        nc.vector.memset(junk, 0.01)
        with nc.named_scope("warmup"):
            for u in range(3):
                pj = psum.tile([P, SC], F32, tag="fill", bufs=2, name="pj")
                for r in range(16):
                    nc.tensor.matmul(
                        pj, lhsT=junk[:, 0:128], rhs=junk[:, 128:640],
                        start=(r == 0), stop=(r == 15),
                    )
                sink = rope.tile([1, SC], BF16, tag="sink", bufs=2, name="sink")
                nc.vector.tensor_copy(sink, pj[0:1, :])

        def gen_proj_schunk(s):
            """Emit s-chunk s projections + RoPE + v as units (yields).

            Unit order (k, q0, v x4, q1-q3) so an attention chunk's pg0
            dependencies (k, qkrot[0], vnat) complete first. Matmul
            groups stay consecutive and eviction chains (rope / v
            copies) run on DVE/DMA only, so cross-engine releases never
            head-of-line-block the PE stream for long."""

            def m_unit(m):
                ps = psum.tile([P, SC], F32, tag="fill", bufs=2, name="ps_proj")
                for h0 in (0, 8):
                    for h in range(h0, h0 + 8):
                        nc.tensor.matmul(
                            ps,
                            lhsT=wsl(h, m * P, (m + 1) * P),
                            rhs=xsl(h, s * SC, (s + 1) * SC),
                            start=(h == 0),
                            stop=(h == NHID - 1),
                        )
                    yield
                # RoPE: q*cos + swap(q*sin2) where sin2 pre-folds the
                # rotate-half signs; the +-32-partition swap must go
                # through DMA (engines are lane-locked)
                sl = slice(s * SC, (s + 1) * SC)
                t1 = rope.tile([P, SC], BF16, tag="t1", bufs=2, name="t1")
                nc.vector.tensor_mul(t1, ps, cos_sb[:, sl])
                tmp = rope.tile([P, SC], BF16, tag="tmp", bufs=2, name="tmp")
                nc.vector.tensor_mul(tmp, ps, sin2_sb[:, sl])
                tswp = rope.tile([P, SC], BF16, tag="tswp", bufs=2, name="tswp")
                for dst, src in ((0, 32), (32, 0), (64, 96), (96, 64)):
                    nc.sync.dma_start(
                        out=tswp[dst:dst + 32, :], in_=tmp[src:src + 32, :]
                    )
                nc.vector.tensor_add(qkrot[m][:, sl], t1, tswp)
                yield

            def v_unit(sb):
                t = vnat[sb]
                nc.vector.memset(t[:, 64:65], 1.0)
                nc.vector.memset(t[:, 65:66], 0.0)
                nc.vector.memset(t[:, 130:131], 1.0)
                nc.vector.memset(t[:, 131:194], 0.0)
                pv = psum.tile([P, 128], F32, tag="fill", bufs=2, name="ps_v")
                for h in range(NHID):
                    nc.tensor.matmul(
                        pv,
                        lhsT=xsl(h, sb * P, (sb + 1) * P),
                        rhs=wsl(h, 640, 768),
                        start=(h == 0),
                        stop=(h == NHID - 1),
                    )
                yield
                nc.vector.tensor_copy(t[:, 0:64], pv[:, 0:64])
                nc.vector.tensor_copy(t[:, 66:130], pv[:, 64:128])
                yield

            yield from m_unit(4)
            yield from m_unit(0)
            for sb in range(4 * s, 4 * s + 4):
                yield from v_unit(sb)
            for m in (1, 2, 3):
                yield from m_unit(m)

        odve = [True]

        def gen_o_chunk(c):
            for qb in range(4 * c, 4 * c + 4):
                ob = obp.tile([P, HID], BF16, tag="ob", name="ob")
                for n in range(HID // SC):
                    po = psum.tile([P, SC], F32, tag="fill", bufs=2, name="po")
                    for i in range(4):
                        nc.tensor.matmul(
                            po,
                            lhsT=attnT[i][:, qb * P:(qb + 1) * P],
                            rhs=wo_sb[i][:, n * SC:(n + 1) * SC],
                            start=(i == 0),
                            stop=(i == 3),
                        )
                    # alternate the PSUM->SBUF evict between DVE and ACT
                    # to balance the two engines' attention-phase load
                    if odve[0]:
                        nc.vector.tensor_copy(ob[:, n * SC:(n + 1) * SC], po)
                    else:
                        nc.scalar.copy(ob[:, n * SC:(n + 1) * SC], po)
                    odve[0] = not odve[0]
                    yield
                nc.sync.dma_start(out=o_part[qb * P:(qb + 1) * P, :], in_=ob)

        def gen_norm(av0, av1, pg, q0):
            """Normalize one pg: stage AV to SBUF (releases the av banks),
            reciprocal of the ones-column denominators (row 64), broadcast
            (the ONLY gpsimd op - keeps its IRAM kernel resident), scale on
            DVE. attnT rows 0-63 = head pg, 64-127 = head pg+4 (cross-
            partition move via DMA). Emitted as quanta inside the NEXT
            pg's kb loop so the av-bank release never stalls the PE."""
            st01 = stg.tile([65, 2 * SC], F32, tag="stage", name="st01")
            nc.vector.tensor_copy(st01[:, 0:SC], av0[0:65, :])
            yield
            nc.vector.tensor_copy(st01[:, SC:2 * SC], av1[0:65, :])
            yield
            den2 = stg.tile([1, 2 * SC], F32, tag="den", bufs=1, name="den2")
            nc.scalar.dma_start(out=den2, in_=st01[64:65, :])
            rec = stg.tile([1, 2 * SC], F32, tag="rec", bufs=1, name="rec")
            nc.vector.reciprocal_approx_fast(rec, den2)
            yield
            rb = stg.tile([64, 2 * SC], F32, tag="rb", bufs=1, name="rb")
            nc.gpsimd.partition_broadcast(rb, rec)
            yield
            nc.vector.tensor_mul(
                attnT[pg][0:64, q0:q0 + SC], st01[0:64, 0:SC], rb[:, 0:SC]
            )
            yield
            hi = stg.tile([64, SC], BF16, tag="hi", name="hi")
            nc.vector.tensor_mul(hi, st01[0:64, SC:2 * SC], rb[:, SC:2 * SC])
            nc.scalar.dma_start(out=attnT[pg][64:128, q0:q0 + SC], in_=hi)
            yield

        # ---- s-chunk 0 projections run dense (nothing to overlap yet) ----
        with nc.named_scope("projA0"):
            for _ in gen_proj_schunk(0):
                pass

        # ---- attention chunks; chunk c's kb loops consume proj-s(c+1)
        # and o_proj(c-1) units as PE filler so the PE never idles on the
        # ACT exp stream; proj units drain by chunk end (next chunk needs
        # their qkrot/vnat), o units may spill into the next chunk ----
        o_gen = None
        norm_q = []
        for c in range(NSC):
          with nc.named_scope(f"attn_c{c}"):
            q0 = c * SC
            nkb = 4 * c + 4
            p_gen = gen_proj_schunk(c + 1) if c + 1 < NSC else None
            # all pending normalizes belong to chunk c-1, whose attnT the
            # o_proj(c-1) units read: flush them before o units can be
            # emitted (they overlap the proj drain / chunk entry)
            for g in norm_q:
                for _ in g:
                    pass
            norm_q = []
            # o(0) feeds chunk 1; o(1)+o(2) are deferred to chunk 3,
            # which is otherwise ACT-bound while chunks 1-2 are PE-bound
            if c == 1:
                o_gen = gen_o_chunk(0)
            elif c == NSC - 1:
                for cc in range(1, NSC - 1):
                    o_prev = gen_o_chunk(cc)
                    o_gen = o_prev if o_gen is None else _chain(o_gen, o_prev)
            take_p = [True]

            def take_filler(k=1):
                nonlocal p_gen, o_gen
                for _ in range(k):
                    if take_p[0] and p_gen is not None:
                        try:
                            next(p_gen)
                        except StopIteration:
                            p_gen = None
                    elif o_gen is not None:
                        try:
                            next(o_gen)
                        except StopIteration:
                            o_gen = None
                    elif p_gen is not None:
                        try:
                            next(p_gen)
                        except StopIteration:
                            p_gen = None
                    else:
                        return
                    take_p[0] = not take_p[0]

            for pg in (0, 1, 2, 3):
                av0 = psum.tile([P, SC], F32, tag="av", bufs=2, name="av0")
                av1 = psum.tile([P, SC], F32, tag="av", bufs=2, name="av1")

                def emit_av(kb, pt, vs):
                    nc.tensor.matmul(
                        av0[:, vs:SC],
                        lhsT=vnat[kb][:, 0:128],
                        rhs=pt[:, vs:SC],
                        start=(kb == 0), stop=(kb == nkb - 1),
                    )
                    nc.tensor.matmul(
                        av1[:, vs:SC],
                        lhsT=vnat[kb][:, 66:194],
                        rhs=pt[:, SC:2 * SC - vs],
                        start=(kb == 0), stop=(kb == nkb - 1),
                    )

                # software pipeline: AV(kb-3) is emitted after scores(kb),
                # giving each exp three iterations of cover
                pending = []
                for kb in range(nkb):
                    # spill the previous pg's normalize ops in here
                    while norm_q and norm_q[0] is not None:
                        try:
                            next(norm_q[0])
                            break
                        except StopIteration:
                            norm_q.pop(0)
                    vs = max(0, (kb - 4 * c) * P)  # first valid col in chunk
                    st = psum.tile([P, 2 * SC], F32, tag="st", bufs=2, name="st")
                    nc.tensor.matmul(
                        st[:, vs:SC],
                        lhsT=qkrot[4][0:64, kb * P:(kb + 1) * P],
                        rhs=qkrot[pg][0:64, q0 + vs:q0 + SC],
                        start=True, stop=True,
                    )
                    # head hp+4 written left-shifted by vs so the exp span
                    # [vs : 2SC-vs] is contiguous with no dead columns
                    nc.tensor.matmul(
                        st[:, SC:2 * SC - vs],
                        lhsT=qkrot[4][64:128, kb * P:(kb + 1) * P],
                        rhs=qkrot[pg][64:128, q0 + vs:q0 + SC],
                        start=True, stop=True,
                    )
                    if len(pending) >= 3:
                        emit_av(*pending.pop(0))
                    pt = ptp.tile([P, 2 * SC], BF16, tag="pt", name="pt")
                    nc.scalar.activation(
                        pt[:, vs:2 * SC - vs], st[:, vs:2 * SC - vs], EXP,
                        scale=0.125,
                    )
                    if kb - 4 * c >= 0:  # diagonal block: mask triangle
                        nc.vector.tensor_mul(
                            pt[:, vs:vs + P], pt[:, vs:vs + P], mask_sb
                        )
                        nc.vector.tensor_mul(
                            pt[:, SC:SC + P], pt[:, SC:SC + P], mask_sb
                        )
                    pending.append((kb, pt, vs))
                    take_filler(2)
                for pp in pending:
                    emit_av(*pp)
                    take_filler(1)
                norm_q.append(gen_norm(av0, av1, pg, q0))

            # drain remaining proj units + spilled normalizes
            while p_gen is not None or norm_q:
                if norm_q:
                    try:
                        next(norm_q[0])
                    except StopIteration:
                        norm_q.pop(0)
                if p_gen is not None:
                    try:
                        next(p_gen)
                    except StopIteration:
                        p_gen = None
                if c == NSC - 1 and p_gen is None:
                    # final chunk: flush all normalizes before the o tail
                    for g in norm_q:
                        for _ in g:
                            pass
                    norm_q = []
        # last chunk's o_proj tail (+ any o units that spilled over)
        if o_gen is not None:
            for _ in o_gen:
                pass
        for _ in gen_o_chunk(NSC - 1):
            pass

    nc.finalize()
    return nc


def _chain(g1, g2):
    yield from g1
    yield from g2


def prep_core_inputs(x, cos, sin, wq, wk, wv, wo, core, _shared={}):
    """Build the per-core input map (all host-side numpy)."""
    b, g = core // 4, core % 4
    S = x.shape[1]

    key = ("xT", b, id(x))
    if key not in _shared:
        _shared.clear() if len(_shared) > 8 else None
        _shared[key] = np.ascontiguousarray(x[b].T).astype(NP_BF16)
    xT = _shared[key]

    qcols = []
    for i in range(4):
        h0, h1 = 8 * g + i, 8 * g + i + 4
        qcols.append(wq[:, h0 * D:(h0 + 1) * D])
        qcols.append(wq[:, h1 * D:(h1 + 1) * D])
    kcols = wk[:, 2 * g * D:(2 * g + 2) * D]
    vcols = wv[:, 2 * g * D:(2 * g + 2) * D]
    wqkv_c = np.concatenate(qcols + [kcols, vcols], axis=1).astype(NP_BF16)
    worows = []
    for i in range(4):
        h0, h1 = 8 * g + i, 8 * g + i + 4
        worows.append(wo[h0 * D:(h0 + 1) * D, :])
        worows.append(wo[h1 * D:(h1 + 1) * D, :])
    wo_c = np.concatenate(worows, axis=0).astype(NP_BF16)

    cosT = np.tile(cos[:S].T, (2, 1)).astype(NP_BF16)
    # sin pre-arranged for multiply-then-swap rope:
    # tmp[j] = q[j]*sin2[j]; result[i] = tmp[swap(i)] needs
    # sin2 = [sin[D/2:], -sin[:D/2]] per 64-row head block
    sinT = sin[:S].T
    sinT2_h = np.concatenate([sinT[D // 2:], -sinT[:D // 2]], axis=0)
    sinT2 = np.tile(sinT2_h, (2, 1)).astype(NP_BF16)
    trimask = np.triu(np.ones((P, P), dtype=NP_BF16))

    return {
        "xT": xT, "wqkv": wqkv_c, "wo": wo_c,
        "cosT": cosT, "sinT2": sinT2, "trimask": trimask,
    }


def kernel(x, cos, sin, wq, wk, wv, wo):
    x = np.asarray(x)
    S = x.shape[1]
    assert x.shape == (B, S, HID)
    if S not in _CACHE:
        _CACHE[S] = build_nc(S)
    nc = _CACHE[S]
    in_maps = [
        prep_core_inputs(x, np.asarray(cos), np.asarray(sin), np.asarray(wq),
                         np.asarray(wk), np.asarray(wv), np.asarray(wo), core)
        for core in range(8)
    ]
    res = run_bass_kernel_spmd(nc, in_maps, core_ids=list(range(8)))
    out = np.zeros((B, S, HID), np.float32)
    for core in range(8):
        out[core // 4] += res.results[core]["o_part"].astype(np.float32)
    return out


# revision 25
# speedup vs baseline: 1.0831x; 1.0797x over previous
"""GQA attention (RoPE + causal softmax + o_proj) on 8 Trainium2 NeuronCores.

Sharding: core = b*4 + g where b = batch (2), g = head-group (4).
Each core handles 8 query heads (global 8g..8g+7) and their 2 KV heads
(2g, 2g+1) for one batch element, producing a partial o_proj output
(contraction over its 512 of the 2048 hd dims). The host sums the 4
partials per batch element.

v3 schedule: projections for s-chunk 0 run dense at the head; the
projections for s-chunks 1-3 and the o_proj of chunk c-1 are fed into
chunk c's kb loop as PE filler quanta, so the PE (the overall pacer at
~226us of stream time) never waits on the ACT exp stream. PSUM is
partitioned into dedicated rings (scores 4 banks / AV 2 / filler 2) so
long-lived AV accumulators never block filler allocation. Softmax
normalization is emitted as quanta into the NEXT pg's kb loop, so the
AV-bank release (an SBUF staging copy) never stalls the PE.

Engine discipline learned from traces:
  - every dma_start costs ~600ns of serial Sync-engine issue time and
    the SP executes in emission order, so the bulk input load uses 9
    mega-DMAs with 3D access patterns (all 16 hid-chunks per
    instruction) instead of ~70 per-tile DMAs that head-block the
    rope-swap/normalize DMAs;
  - GPSIMD runs ONLY partition_broadcast: mixing other ops forces a
    MODIFY_POOL_CONFIG IRAM reload (~6us, invisible in profiles) per
    switch, which serialized every pg at ~12us in an earlier rev;
  - strided-partition DMA views silently corrupt (only one row per
    block lands) - the rope swap stays 4 plain 32-row DMAs;
  - a junk-matmul warmup burst at t=0 flips the PE HAM clock gate to
    2.4 GHz before the first real matmul group's DMA gate clears.
"""

import numpy as np
import ml_dtypes
from contextlib import ExitStack

import concourse.mybir as mybir
from concourse import bacc
from concourse.tile import TileContext
from concourse.bass_utils import run_bass_kernel_spmd

BF16 = mybir.dt.bfloat16
F32 = mybir.dt.float32
NP_BF16 = ml_dtypes.bfloat16

HID = 2048
D = 64
H = 32           # global query heads
KV = 8           # global kv heads
B = 2
P = 128
SC = 512         # q-chunk width (also matmul free dim / PSUM bank)

_CACHE = {}


def build_nc(S):
    assert S % SC == 0
    NHID = HID // P       # hid chunks (16)
    NSB = S // P          # 128-row s-blocks
    NSC = S // SC         # 512-col s-chunks
    EXP = mybir.ActivationFunctionType.Exp

    nc = bacc.Bacc("TRN2", target_bir_lowering=False, debug=False)
    xT = nc.dram_tensor("xT", [HID, S], BF16, kind="ExternalInput")
    # [q pairs (512) | k (128) | v (128)] merged per hid chunk
    wqkv = nc.dram_tensor("wqkv", [HID, 768], BF16, kind="ExternalInput")
    wo = nc.dram_tensor("wo", [512, HID], BF16, kind="ExternalInput")
    cosT = nc.dram_tensor("cosT", [128, S], BF16, kind="ExternalInput")
    sinT2 = nc.dram_tensor("sinT2", [128, S], BF16, kind="ExternalInput")
    trimask = nc.dram_tensor("trimask", [128, 128], BF16, kind="ExternalInput")
    o_part = nc.dram_tensor("o_part", [S, HID], BF16, kind="ExternalOutput")

    with TileContext(nc) as tc, ExitStack() as ctx:
        res = ctx.enter_context(tc.tile_pool(name="res", bufs=1))
        rope = ctx.enter_context(tc.tile_pool(name="rope", bufs=2))
        ptp = ctx.enter_context(tc.tile_pool(name="ptp", bufs=6))
        stg = ctx.enter_context(tc.tile_pool(name="stg", bufs=2))
        obp = ctx.enter_context(tc.tile_pool(name="obp", bufs=2))
        psum = ctx.enter_context(tc.tile_pool(name="psum", bufs=1, space="PSUM"))

        # ---- mega-tiles: all 16 hid-chunks side by side ----
        wqall = res.tile([P, NHID * 768], BF16, tag="wqall")
        xtall = res.tile([P, NHID * S], BF16, tag="xtall")

        def wsl(h, lo, hi):
            return wqall[:, h * 768 + lo:h * 768 + hi]

        def xsl(h, lo, hi):
            return xtall[:, h * S + lo:h * S + hi]

        wqv = wqall.rearrange("p (h c) -> p h c", c=768)
        wqs = wqkv.rearrange("(h p) c -> p h c", h=NHID)
        xtv = xtall.rearrange("p (h c) -> p h c", c=S)
        xts_ = xT.rearrange("(h p) c -> p h c", h=NHID)

        # ---- input staging: mega-DMAs ordered by first consumption.
        # Each issuing ENGINE owns a hardware DGE queue (issue and
        # transfer serialize per queue), so the loads are split between
        # the sync and scalar queues for parallel transfer streams, and
        # runtime DMAs never queue behind bulk loads. ----
        nc.sync.dma_start(out=xtv[:, :, 0:SC], in_=xts_[:, :, 0:SC])
        nc.sync.dma_start(out=wqv[:, :, 0:128], in_=wqs[:, :, 0:128])
        nc.scalar.dma_start(out=wqv[:, :, 512:768], in_=wqs[:, :, 512:768])
        cos_sb = res.tile([P, S], BF16, tag="cos")
        nc.scalar.dma_start(out=cos_sb, in_=cosT[:, :])
        sin2_sb = res.tile([P, S], BF16, tag="sin2")
        nc.scalar.dma_start(out=sin2_sb, in_=sinT2[:, :])
        mask_sb = res.tile([P, P], BF16, tag="mask")
        nc.scalar.dma_start(out=mask_sb, in_=trimask[:, :])
        nc.scalar.dma_start(out=wqv[:, :, 128:512], in_=wqs[:, :, 128:512])
        nc.scalar.dma_start(out=xtv[:, :, SC:2 * SC], in_=xts_[:, :, SC:2 * SC])
        wo_sb = []
        for i in range(4):
            t = res.tile([P, HID], BF16, tag=f"wo{i}", name=f"wo{i}")
            nc.scalar.dma_start(out=t, in_=wo[i * P:(i + 1) * P, :])
            wo_sb.append(t)
        nc.scalar.dma_start(out=xtv[:, :, 2 * SC:4 * SC], in_=xts_[:, :, 2 * SC:4 * SC])

        # ---- persistent compute tiles ----
        qkrot = [res.tile([P, S], BF16, tag=f"qkrot{m}", name=f"qkrot{m}")
                 for m in range(5)]
        # v tiles [128, 194]: [v0(0:64) | 1 | 0 | v1(66:130) | 1 | 0-pad] —
        # both AV stationaries are 128 columns (FWL) at 4B-aligned offsets;
        # the ones column puts the softmax denominator in PSUM row 64
        vnat = [res.tile([P, 194], BF16, tag=f"vnat{sb}", name=f"vnat{sb}")
                for sb in range(NSB)]
        attnT = [res.tile([P, S], BF16, tag=f"attnT{i}", name=f"attnT{i}")
                 for i in range(4)]

        # ---- PE warmup: flip the HAM clock gate to 8/8 while the input
        # DMAs stream; junk matmuls depend only on one vector memset ----
        junk = res.tile([P, 640], BF16, tag="junk")
        nc.vector.memset(junk, 0.01)
        ones1 = res.tile([1, 64], BF16, tag="ones1")
        nc.vector.memset(ones1, 1.0)
        with nc.named_scope("warmup"):
            for u in range(3):
                pj = psum.tile([P, SC], F32, tag="fill", bufs=2, name="pj")
                for r in range(16):
                    nc.tensor.matmul(
                        pj, lhsT=junk[:, 0:128], rhs=junk[:, 128:640],
                        start=(r == 0), stop=(r == 15),
                    )
                sink = rope.tile([1, SC], BF16, tag="sink", bufs=2, name="sink")
                nc.vector.tensor_copy(sink, pj[0:1, :])

        def gen_proj_schunk(s):
            """Emit s-chunk s projections + RoPE + v as units (yields).

            Unit order (k, q0, v x4, q1-q3) so an attention chunk's pg0
            dependencies (k, qkrot[0], vnat) complete first. Matmul
            groups stay consecutive and eviction chains (rope / v
            copies) run on DVE/DMA only, so cross-engine releases never
            head-of-line-block the PE stream for long."""

            def m_unit(m):
                ps = psum.tile([P, SC], F32, tag="fill", bufs=2, name="ps_proj")
                for h0 in (0, 8):
                    for h in range(h0, h0 + 8):
                        nc.tensor.matmul(
                            ps,
                            lhsT=wsl(h, m * P, (m + 1) * P),
                            rhs=xsl(h, s * SC, (s + 1) * SC),
                            start=(h == 0),
                            stop=(h == NHID - 1),
                        )
                    yield
                # RoPE: q*cos + swap(q*sin2) where sin2 pre-folds the
                # rotate-half signs; the +-32-partition swap must go
                # through DMA (engines are lane-locked)
                sl = slice(s * SC, (s + 1) * SC)
                t1 = rope.tile([P, SC], BF16, tag="t1", bufs=2, name="t1")
                nc.vector.tensor_mul(t1, ps, cos_sb[:, sl])
                tmp = rope.tile([P, SC], BF16, tag="tmp", bufs=2, name="tmp")
                nc.vector.tensor_mul(tmp, ps, sin2_sb[:, sl])
                tswp = rope.tile([P, SC], BF16, tag="tswp", bufs=2, name="tswp")
                for dst, src in ((0, 32), (32, 0), (64, 96), (96, 64)):
                    nc.sync.dma_start(
                        out=tswp[dst:dst + 32, :], in_=tmp[src:src + 32, :]
                    )
                nc.vector.tensor_add(qkrot[m][:, sl], t1, tswp)
                yield

            def v_unit(sb):
                t = vnat[sb]
                nc.vector.memset(t[:, 64:65], 1.0)
                nc.vector.memset(t[:, 65:66], 0.0)
                nc.vector.memset(t[:, 130:131], 1.0)
                nc.vector.memset(t[:, 131:194], 0.0)
                pv = psum.tile([P, 128], F32, tag="fill", bufs=2, name="ps_v")
                for h in range(NHID):
                    nc.tensor.matmul(
                        pv,
                        lhsT=xsl(h, sb * P, (sb + 1) * P),
                        rhs=wsl(h, 640, 768),
                        start=(h == 0),
                        stop=(h == NHID - 1),
                    )
                yield
                nc.vector.tensor_copy(t[:, 0:64], pv[:, 0:64])
                nc.vector.tensor_copy(t[:, 66:130], pv[:, 64:128])
                yield

            yield from m_unit(4)
            yield from m_unit(0)
            for sb in range(4 * s, 4 * s + 4):
                yield from v_unit(sb)
            for m in (1, 2, 3):
                yield from m_unit(m)

        odve = [True]

        def gen_o_chunk(c):
            for qb in range(4 * c, 4 * c + 4):
                ob = obp.tile([P, HID], BF16, tag="ob", name="ob")
                for n in range(HID // SC):
                    po = psum.tile([P, SC], F32, tag="fill", bufs=2, name="po")
                    for i in range(4):
                        nc.tensor.matmul(
                            po,
                            lhsT=attnT[i][:, qb * P:(qb + 1) * P],
                            rhs=wo_sb[i][:, n * SC:(n + 1) * SC],
                            start=(i == 0),
                            stop=(i == 3),
                        )
                    # alternate the PSUM->SBUF evict between DVE and ACT
                    # to balance the two engines' attention-phase load
                    if odve[0]:
                        nc.vector.tensor_copy(ob[:, n * SC:(n + 1) * SC], po)
                    else:
                        nc.scalar.copy(ob[:, n * SC:(n + 1) * SC], po)
                    odve[0] = not odve[0]
                    yield
                nc.sync.dma_start(out=o_part[qb * P:(qb + 1) * P, :], in_=ob)

        def gen_norm(av0, av1, pg, q0):
            """Normalize one pg: stage AV to SBUF (releases the av banks),
            reciprocal of the ones-column denominators (row 64), broadcast
            (the ONLY gpsimd op - keeps its IRAM kernel resident), scale on
            DVE. attnT rows 0-63 = head pg, 64-127 = head pg+4 (cross-
            partition move via DMA). Emitted as quanta inside the NEXT
            pg's kb loop so the av-bank release never stalls the PE."""
            st01 = stg.tile([65, 2 * SC], F32, tag="stage", name="st01")
            nc.vector.tensor_copy(st01[:, 0:SC], av0[0:65, :])
            yield
            nc.vector.tensor_copy(st01[:, SC:2 * SC], av1[0:65, :])
            yield
            den2 = stg.tile([1, 2 * SC], F32, tag="den", bufs=1, name="den2")
            nc.gpsimd.dma_start(out=den2, in_=st01[64:65, :])
            rec = stg.tile([1, 2 * SC], F32, tag="rec", bufs=1, name="rec")
            nc.vector.reciprocal_approx_fast(rec, den2)
            yield
            rb = stg.tile([64, 2 * SC], F32, tag="rb", bufs=1, name="rb")
            nc.gpsimd.partition_broadcast(rb, rec)
            yield
            nc.vector.tensor_mul(
                attnT[pg][0:64, q0:q0 + SC], st01[0:64, 0:SC], rb[:, 0:SC]
            )
            yield
            hi = stg.tile([64, SC], BF16, tag="hi", name="hi")
            nc.vector.tensor_mul(hi, st01[0:64, SC:2 * SC], rb[:, SC:2 * SC])
            nc.gpsimd.dma_start(out=attnT[pg][64:128, q0:q0 + SC], in_=hi)
            yield

        def norm_final(av0, av1, pg, q0):
            """Tail fast path (last pg of the last chunk): broadcast the
            denominators via two tiny PE matmuls against a ones column
            (the PE is idle here; gpsimd's broadcast costs ~3.4us) and
            scale straight out of the AV PSUM banks."""
            den2 = stg.tile([1, 2 * SC], BF16, tag="den", bufs=1, name="den2f")
            nc.vector.tensor_copy(den2[:, 0:SC], av0[64:65, :])
            nc.vector.tensor_copy(den2[:, SC:2 * SC], av1[64:65, :])
            rb0 = psum.tile([64, SC], F32, tag="fill", bufs=2, name="rb0")
            nc.tensor.matmul(rb0, lhsT=ones1, rhs=den2[:, 0:SC],
                             start=True, stop=True)
            rb1 = psum.tile([64, SC], F32, tag="fill", bufs=2, name="rb1")
            nc.tensor.matmul(rb1, lhsT=ones1, rhs=den2[:, SC:2 * SC],
                             start=True, stop=True)
            rec01 = stg.tile([64, 2 * SC], F32, tag="rb", bufs=1, name="rec01")
            nc.vector.reciprocal_approx_fast(rec01[:, 0:SC], rb0)
            nc.vector.reciprocal_approx_fast(rec01[:, SC:2 * SC], rb1)
            nc.vector.tensor_mul(
                attnT[pg][0:64, q0:q0 + SC], av0[0:64, :], rec01[:, 0:SC]
            )
            hi = stg.tile([64, SC], BF16, tag="hi", name="hif")
            nc.vector.tensor_mul(hi, av1[0:64, :], rec01[:, SC:2 * SC])
            nc.gpsimd.dma_start(out=attnT[pg][64:128, q0:q0 + SC], in_=hi)

        # ---- s-chunk 0 projections run dense (nothing to overlap yet) ----
        with nc.named_scope("projA0"):
            for _ in gen_proj_schunk(0):
                pass

        # ---- attention chunks; chunk c's kb loops consume proj-s(c+1)
        # and o_proj(c-1) units as PE filler so the PE never idles on the
        # ACT exp stream; proj units drain by chunk end (next chunk needs
        # their qkrot/vnat), o units may spill into the next chunk ----
        o_gen = None
        norm_q = []
        for c in range(NSC):
          with nc.named_scope(f"attn_c{c}"):
            q0 = c * SC
            nkb = 4 * c + 4
            p_gen = gen_proj_schunk(c + 1) if c + 1 < NSC else None
            # all pending normalizes belong to chunk c-1, whose attnT the
            # o_proj(c-1) units read: flush them before o units can be
            # emitted (they overlap the proj drain / chunk entry)
            for g in norm_q:
                for _ in g:
                    pass
            norm_q = []
            # o(0) feeds chunk 1; o(1)+o(2) are deferred to chunk 3,
            # which is otherwise ACT-bound while chunks 1-2 are PE-bound
            if c == 1:
                o_gen = gen_o_chunk(0)
            elif c == NSC - 1:
                for cc in range(1, NSC - 1):
                    o_prev = gen_o_chunk(cc)
                    o_gen = o_prev if o_gen is None else _chain(o_gen, o_prev)
            take_p = [True]
            # even pacing over the chunk's kb iterations: frontloading
            # exhausts the fillers early and leaves the late kbs PE-idle
            # on the ACT exp stream (proj gen = 23 quanta, o gen = 16)
            n_avail = (23 if p_gen is not None else 0) + \
                (16 if c == 1 else 32 if c == NSC - 1 else 0)
            total_iters = 4 * nkb
            spent = [0]
            it_ctr = [0]

            def take_filler(k=1):
                nonlocal p_gen, o_gen
                for _ in range(k):
                    if take_p[0] and p_gen is not None:
                        try:
                            next(p_gen)
                        except StopIteration:
                            p_gen = None
                    elif o_gen is not None:
                        try:
                            next(o_gen)
                        except StopIteration:
                            o_gen = None
                    elif p_gen is not None:
                        try:
                            next(p_gen)
                        except StopIteration:
                            p_gen = None
                    else:
                        return
                    spent[0] += 1
                    take_p[0] = not take_p[0]

            def take_paced():
                it_ctr[0] += 1
                want = (it_ctr[0] * n_avail + total_iters - 1) // total_iters
                if spent[0] < want:
                    take_filler(want - spent[0])

            for pg in (0, 1, 2, 3):
                av0 = psum.tile([P, SC], F32, tag="av", bufs=2, name="av0")
                av1 = psum.tile([P, SC], F32, tag="av", bufs=2, name="av1")

                def emit_av(kb, pt, vs):
                    nc.tensor.matmul(
                        av0[:, vs:SC],
                        lhsT=vnat[kb][:, 0:128],
                        rhs=pt[:, vs:SC],
                        start=(kb == 0), stop=(kb == nkb - 1),
                    )
                    nc.tensor.matmul(
                        av1[:, vs:SC],
                        lhsT=vnat[kb][:, 66:194],
                        rhs=pt[:, SC:2 * SC - vs],
                        start=(kb == 0), stop=(kb == nkb - 1),
                    )

                # software pipeline: AV(kb-3) is emitted after scores(kb),
                # giving each exp three iterations of cover
                pending = []
                for kb in range(nkb):
                    # spill the previous pg's normalize ops in here
                    while norm_q and norm_q[0] is not None:
                        try:
                            next(norm_q[0])
                            break
                        except StopIteration:
                            norm_q.pop(0)
                    vs = max(0, (kb - 4 * c) * P)  # first valid col in chunk
                    st = psum.tile([P, 2 * SC], F32, tag="st", bufs=2, name="st")
                    nc.tensor.matmul(
                        st[:, vs:SC],
                        lhsT=qkrot[4][0:64, kb * P:(kb + 1) * P],
                        rhs=qkrot[pg][0:64, q0 + vs:q0 + SC],
                        start=True, stop=True,
                    )
                    # head hp+4 written left-shifted by vs so the exp span
                    # [vs : 2SC-vs] is contiguous with no dead columns
                    nc.tensor.matmul(
                        st[:, SC:2 * SC - vs],
                        lhsT=qkrot[4][64:128, kb * P:(kb + 1) * P],
                        rhs=qkrot[pg][64:128, q0 + vs:q0 + SC],
                        start=True, stop=True,
                    )
                    if len(pending) >= 3:
                        emit_av(*pending.pop(0))
                    pt = ptp.tile([P, 2 * SC], BF16, tag="pt", name="pt")
                    nc.scalar.activation(
                        pt[:, vs:2 * SC - vs], st[:, vs:2 * SC - vs], EXP,
                        scale=0.125,
                    )
                    if kb - 4 * c >= 0:  # diagonal block: mask triangle
                        nc.vector.tensor_mul(
                            pt[:, vs:vs + P], pt[:, vs:vs + P], mask_sb
                        )
                        nc.vector.tensor_mul(
                            pt[:, SC:SC + P], pt[:, SC:SC + P], mask_sb
                        )
                    pending.append((kb, pt, vs))
                    take_paced()
                for pp in pending:
                    emit_av(*pp)
                    take_filler(1)
                if c == NSC - 1 and pg == 3:
                    norm_final(av0, av1, pg, q0)
                else:
                    norm_q.append(gen_norm(av0, av1, pg, q0))

            # drain remaining proj units + spilled normalizes
            while p_gen is not None or norm_q:
                if norm_q:
                    try:
                        next(norm_q[0])
                    except StopIteration:
                        norm_q.pop(0)
                if p_gen is not None:
                    try:
                        next(p_gen)
                    except StopIteration:
                        p_gen = None
                if c == NSC - 1 and p_gen is None:
                    # final chunk: flush all normalizes before the o tail
                    for g in norm_q:
                        for _ in g:
                            pass
                    norm_q = []
        # last chunk's o_proj tail (+ any o units that spilled over)
        if o_gen is not None:
            for _ in o_gen:
                pass
        for _ in gen_o_chunk(NSC - 1):
            pass

    nc.finalize()
    return nc


def _chain(g1, g2):
    yield from g1
    yield from g2


def prep_core_inputs(x, cos, sin, wq, wk, wv, wo, core, _shared={}):
    """Build the per-core input map (all host-side numpy)."""
    b, g = core // 4, core % 4
    S = x.shape[1]

    key = ("xT", b, id(x))
    if key not in _shared:
        _shared.clear() if len(_shared) > 8 else None
        _shared[key] = np.ascontiguousarray(x[b].T).astype(NP_BF16)
    xT = _shared[key]

    qcols = []
    for i in range(4):
        h0, h1 = 8 * g + i, 8 * g + i + 4
        qcols.append(wq[:, h0 * D:(h0 + 1) * D])
        qcols.append(wq[:, h1 * D:(h1 + 1) * D])
    kcols = wk[:, 2 * g * D:(2 * g + 2) * D]
    vcols = wv[:, 2 * g * D:(2 * g + 2) * D]
    wqkv_c = np.concatenate(qcols + [kcols, vcols], axis=1).astype(NP_BF16)
    worows = []
    for i in range(4):
        h0, h1 = 8 * g + i, 8 * g + i + 4
        worows.append(wo[h0 * D:(h0 + 1) * D, :])
        worows.append(wo[h1 * D:(h1 + 1) * D, :])
    wo_c = np.concatenate(worows, axis=0).astype(NP_BF16)

    cosT = np.tile(cos[:S].T, (2, 1)).astype(NP_BF16)
    # sin pre-arranged for multiply-then-swap rope:
    # tmp[j] = q[j]*sin2[j]; result[i] = tmp[swap(i)] needs
    # sin2 = [sin[D/2:], -sin[:D/2]] per 64-row head block
    sinT = sin[:S].T
    sinT2_h = np.concatenate([sinT[D // 2:], -sinT[:D // 2]], axis=0)
    sinT2 = np.tile(sinT2_h, (2, 1)).astype(NP_BF16)
    trimask = np.triu(np.ones((P, P), dtype=NP_BF16))

    return {
        "xT": xT, "wqkv": wqkv_c, "wo": wo_c,
        "cosT": cosT, "sinT2": sinT2, "trimask": trimask,
    }


def kernel(x, cos, sin, wq, wk, wv, wo):
    x = np.asarray(x)
    S = x.shape[1]
    assert x.shape == (B, S, HID)
    if S not in _CACHE:
        _CACHE[S] = build_nc(S)
    nc = _CACHE[S]
    in_maps = [
        prep_core_inputs(x, np.asarray(cos), np.asarray(sin), np.asarray(wq),
                         np.asarray(wk), np.asarray(wv), np.asarray(wo), core)
        for core in range(8)
    ]
    res = run_bass_kernel_spmd(nc, in_maps, core_ids=list(range(8)))
    out = np.zeros((B, S, HID), np.float32)
    for core in range(8):
        out[core // 4] += res.results[core]["o_part"].astype(np.float32)
    return out
